# revision 1
# baseline (speedup 1.0000x reference)
"""Pairwise Euclidean distance matrix on 8 Trainium2 NeuronCores.

Problem: mapping [8192, 512] f32 -> out[i,j] = ||mapping_i - mapping_j||_2,
shape [8192, 8192] f32.

Strategy (row/data parallel, per the sharding hint): core c computes output
rows [c*1024, (c+1)*1024). Since kernel() receives the full input on host,
each core is fed the full mapping directly (no on-device all-gather needed).

Math: d2 = sq_m + sq_n - 2*G with G = A_c @ A^T, then out = sqrt(max(d2, 0)).
The gram block is computed on TensorE in float32r (full-rate fp32 matmul,
~13-bit multiply mantissa - plenty for this problem); the lhs operand is
pre-scaled by -2 on host so PSUM accumulates -2G directly. DVE adds the two
squared-norm terms (per-partition scalar + broadcast row), clamps at 0, and
ScalarE takes the sqrt.

Host-side prep (layout only): mapping^T contiguous (so the contraction dim
lands on SBUF partitions), the -2-scaled per-core lhs slice, and the row
norms sq laid out both as a [1, N] row and a per-core [128, MT] column table.
"""

import numpy as np
import bass_rust
import concourse.bass as bass
import concourse.mybir as mybir
from concourse.tile import TileContext
from concourse.bass_utils import run_bass_kernel_spmd

N = 8192          # points
D = 512           # dim
NCORES = 8
ROWS = N // NCORES        # 1024 output rows per core
MT = ROWS // 128          # 8 m-tiles (128 rows each)
NTILE = 512               # output columns per tile (one PSUM bank)
NT = N // NTILE           # 16 n-tiles
KC = D // 128             # 4 contraction chunks of 128

F32 = mybir.dt.float32
F32R = mybir.dt.float32r
ADD = mybir.AluOpType.add
MAX = mybir.AluOpType.max


def _split_excess_waits(nc, limit=1):
    """The walrus build in this container rejects instructions carrying more
    than one sem-wait (e.g. fp32r Matmult S3_LW). Hoist excess waits onto
    same-engine NoOps inserted immediately before the instruction - waits
    execute in stream order on the engine's sequencer, so blocking semantics
    are identical."""
    for fn in nc.m.functions:
        for blk in fn.blocks:
            newlist = []
            changed = False
            for ins in blk.instructions:
                si = ins.sync_info
                if si is not None and si.on_wait and len(si.on_wait) > limit:
                    waits = list(si.on_wait)
                    excess, keep = waits[:-limit], waits[-limit:]
                    for i, w in enumerate(excess):
                        nop = bass_rust.InstNoOp(
                            name=f"{ins.name}-wsplit{i}", ins=[], outs=[]
                        )
                        nop.engine = ins.engine
                        nop.sync_info = mybir.SyncInfo(on_wait=[w], on_update=[])
                        newlist.append(nop)
                    si.on_wait = keep
                    ins.sync_info = si
                    changed = True
                newlist.append(ins)
            if changed:
                blk.instructions = newlist


def _build():
    nc = bass.Bass()
    at_d = nc.dram_tensor("at", [D, N], F32R, kind="ExternalInput")      # A^T
    lhs_d = nc.dram_tensor("lhs", [D, ROWS], F32R, kind="ExternalInput")  # -2*A_c^T
    sqr_d = nc.dram_tensor("sqr", [1, N], F32, kind="ExternalInput")
    sqm_d = nc.dram_tensor("sqm", [128, MT], F32, kind="ExternalInput")
    out_d = nc.dram_tensor("out", [ROWS, N], F32, kind="ExternalOutput")

    with TileContext(nc) as tc:
        with (
            tc.tile_pool(name="const", bufs=1) as cpool,
            tc.tile_pool(name="rhs", bufs=2) as rpool,
            tc.tile_pool(name="ps", bufs=4, space="PSUM") as pspool,
            tc.tile_pool(name="t1", bufs=3) as t1pool,
            tc.tile_pool(name="t2", bufs=3) as t2pool,
            tc.tile_pool(name="ot", bufs=4) as opool,
        ):
            # Resident: -2*A_c^T as 4 chunks of [128, ROWS] side by side (2 MB)
            lhs = cpool.tile([128, KC * ROWS], F32R)
            for c in range(KC):
                nc.sync.dma_start(
                    lhs[:, c * ROWS:(c + 1) * ROWS],
                    lhs_d[c * 128:(c + 1) * 128, :],
                )
            # Resident: sq broadcast across partitions [128, N] (4 MB),
            # DMA'd per n-tile slice so the first tiles unblock early.
            sqb = cpool.tile([128, N], F32)
            for n in range(NT):
                nc.sync.dma_start(
                    sqb[:, n * NTILE:(n + 1) * NTILE],
                    sqr_d[0:1, n * NTILE:(n + 1) * NTILE].partition_broadcast(128),
                )
            sqm = cpool.tile([128, MT], F32)
            nc.sync.dma_start(sqm[:], sqm_d[:])

            for n in range(NT):
                ncols = slice(n * NTILE, (n + 1) * NTILE)
                rhs = rpool.tile([128, KC, NTILE], F32R)
                for c in range(KC):
                    nc.sync.dma_start(
                        rhs[:, c, :], at_d[c * 128:(c + 1) * 128, ncols]
                    )
                for m in range(MT):
                    ps = pspool.tile([128, NTILE], F32)
                    for c in range(KC):
                        nc.tensor.matmul(
                            ps[:],
                            lhs[:, c * ROWS + m * 128: c * ROWS + (m + 1) * 128],
                            rhs[:, c, :],
                            start=(c == 0),
                            stop=(c == KC - 1),
                        )
                    # t1 = (-2G + sq_m) + sq_n
                    t1 = t1pool.tile([128, NTILE], F32)
                    nc.vector.scalar_tensor_tensor(
                        t1[:], ps[:], sqm[:, m:m + 1], sqb[:, ncols],
                        op0=ADD, op1=ADD,
                    )
                    # t2 = max(t1, 0)
                    t2 = t2pool.tile([128, NTILE], F32)
                    nc.vector.tensor_scalar_max(t2[:], t1[:], 0.0)
                    # out tile = sqrt(t2)
                    ot = opool.tile([128, NTILE], F32)
                    nc.scalar.sqrt(ot[:], t2[:])
                    nc.sync.dma_start(
                        out_d[m * 128:(m + 1) * 128, ncols], ot[:]
                    )
    _split_excess_waits(nc, limit=1)
    return nc


_NC_CACHE = {}


def kernel(mapping: np.ndarray) -> np.ndarray:
    mapping = np.ascontiguousarray(mapping, dtype=np.float32)
    assert mapping.shape == (N, D)
    at = np.ascontiguousarray(mapping.T)                       # [D, N]
    sq = (
        np.einsum(
            "nd,nd->n", mapping.astype(np.float64), mapping.astype(np.float64)
        ).astype(np.float32)
    )
    sqr = sq.reshape(1, N)

    if "nc" not in _NC_CACHE:
        _NC_CACHE["nc"] = _build()
    nc = _NC_CACHE["nc"]

    in_maps = []
    for c in range(NCORES):
        lhs_c = np.ascontiguousarray(-2.0 * at[:, c * ROWS:(c + 1) * ROWS])
        sqm_c = np.ascontiguousarray(
            sq[c * ROWS:(c + 1) * ROWS].reshape(MT, 128).T
        )  # [128, MT]: [p, m] = sq[c*ROWS + m*128 + p]
        in_maps.append({"at": at, "lhs": lhs_c, "sqr": sqr, "sqm": sqm_c})

    res = run_bass_kernel_spmd(nc, in_maps, core_ids=list(range(NCORES)))
    return np.concatenate(
        [res.results[c]["out"] for c in range(NCORES)], axis=0
    )


# revision 2
# speedup vs baseline: 1.2198x; 1.2198x over previous
"""Pairwise Euclidean distance matrix on 8 Trainium2 NeuronCores.

Problem: mapping [8192, 512] f32 -> out[i,j] = ||mapping_i - mapping_j||_2,
shape [8192, 8192] f32.

Strategy (row/data parallel, per the sharding hint): core c computes output
rows [c*1024, (c+1)*1024). Since kernel() receives the full input on host,
each core is fed the full mapping directly (no on-device all-gather needed).

Math: d2 = sq_m + sq_n - 2*G with G = A_c @ A^T, then out = sqrt(max(d2, 0)).
The gram block is computed on TensorE in float32r (full-rate fp32 matmul,
~13-bit multiply mantissa - plenty here); the lhs operand is pre-scaled by -2
on host so PSUM accumulates -2G directly. DVE adds the two squared-norm terms
(per-partition scalar + broadcast row) and clamps at 0; ScalarE takes sqrt.

Layout: all DMA moves in >=8KB contiguous per-partition lines. A^T lives in
SBUF a quarter (2048 columns) at a time and doubles as the matmul rhs; output
is staged per (quarter, m-tile) in [128, 2048] row buffers.

Host-side prep (layout only): mapping^T contiguous (so the contraction dim
lands on SBUF partitions), the -2-scaled per-core lhs slice, and the row
norms sq laid out both as a [1, N] row and a per-core [128, MT] column table.
"""

import numpy as np
import bass_rust
import concourse.bass as bass
import concourse.mybir as mybir
from concourse.tile import TileContext
from concourse.bass_utils import run_bass_kernel_spmd

N = 8192          # points
D = 512           # dim
NCORES = 8
ROWS = N // NCORES        # 1024 output rows per core
MT = ROWS // 128          # 8 m-tiles (128 rows each)
NTILE = 512               # output columns per matmul (one PSUM bank)
KC = D // 128             # 4 contraction chunks of 128
QCOLS = 2048              # A^T columns resident per quarter
QT = N // QCOLS           # 4 quarters
QN = QCOLS // NTILE       # 4 n-tiles per quarter

F32 = mybir.dt.float32
F32R = mybir.dt.float32r
ADD = mybir.AluOpType.add


def _split_excess_waits(nc, limit=1):
    """The walrus build in this container rejects instructions carrying more
    than one sem-wait (e.g. fp32r Matmult S3_LW). Hoist excess waits onto
    same-engine NoOps inserted immediately before the instruction - waits
    execute in stream order on the engine's sequencer, so blocking semantics
    are identical."""
    for fn in nc.m.functions:
        for blk in fn.blocks:
            newlist = []
            changed = False
            for ins in blk.instructions:
                si = ins.sync_info
                if si is not None and si.on_wait and len(si.on_wait) > limit:
                    waits = list(si.on_wait)
                    excess, keep = waits[:-limit], waits[-limit:]
                    for i, w in enumerate(excess):
                        nop = bass_rust.InstNoOp(
                            name=f"{ins.name}-wsplit{i}", ins=[], outs=[]
                        )
                        nop.engine = ins.engine
                        nop.sync_info = mybir.SyncInfo(on_wait=[w], on_update=[])
                        newlist.append(nop)
                    si.on_wait = keep
                    ins.sync_info = si
                    changed = True
                newlist.append(ins)
            if changed:
                blk.instructions = newlist


def _build():
    nc = bass.Bass()
    at_d = nc.dram_tensor("at", [D, N], F32R, kind="ExternalInput")      # A^T
    lhs_d = nc.dram_tensor("lhs", [D, ROWS], F32R, kind="ExternalInput")  # -2*A_c^T
    sqr_d = nc.dram_tensor("sqr", [1, N], F32, kind="ExternalInput")
    sqm_d = nc.dram_tensor("sqm", [128, MT], F32, kind="ExternalInput")
    out_d = nc.dram_tensor("out", [ROWS, N], F32, kind="ExternalOutput")

    with TileContext(nc) as tc:
        with (
            tc.tile_pool(name="const", bufs=1) as cpool,
            tc.tile_pool(name="atq", bufs=2) as apool,
            tc.tile_pool(name="ps", bufs=4, space="PSUM") as pspool,
            tc.tile_pool(name="t1", bufs=3) as t1pool,
            tc.tile_pool(name="t2", bufs=3) as t2pool,
            tc.tile_pool(name="orow", bufs=3) as opool,
        ):
            # Resident: -2*A_c^T as 4 chunks of [128, ROWS] side by side (2 MB)
            lhs = cpool.tile([128, KC * ROWS], F32R)
            for c in range(KC):
                nc.sync.dma_start(
                    lhs[:, c * ROWS:(c + 1) * ROWS],
                    lhs_d[c * 128:(c + 1) * 128, :],
                )
            # Resident: sq broadcast across partitions [128, N] (4 MB)
            sqb = cpool.tile([128, N], F32)
            for h in range(2):
                nc.sync.dma_start(
                    sqb[:, h * (N // 2):(h + 1) * (N // 2)],
                    sqr_d[0:1, h * (N // 2):(h + 1) * (N // 2)]
                    .partition_broadcast(128),
                )
            sqm = cpool.tile([128, MT], F32)
            nc.sync.dma_start(sqm[:], sqm_d[:])

            for q in range(QT):
                # A^T quarter: 4 chunks of [128, QCOLS] side by side (4 MB),
                # serves directly as the matmul moving operand.
                atq = apool.tile([128, KC * QCOLS], F32R)
                for c in range(KC):
                    nc.sync.dma_start(
                        atq[:, c * QCOLS:(c + 1) * QCOLS],
                        at_d[c * 128:(c + 1) * 128, q * QCOLS:(q + 1) * QCOLS],
                    )
                for m in range(MT):
                    orow = opool.tile([128, QCOLS], F32)
                    for n in range(QN):
                        ns = slice(n * NTILE, (n + 1) * NTILE)
                        ps = pspool.tile([128, NTILE], F32)
                        for c in range(KC):
                            nc.tensor.matmul(
                                ps[:],
                                lhs[:, c * ROWS + m * 128:
                                    c * ROWS + (m + 1) * 128],
                                atq[:, c * QCOLS + n * NTILE:
                                    c * QCOLS + (n + 1) * NTILE],
                                start=(c == 0),
                                stop=(c == KC - 1),
                            )
                        # t1 = (-2G + sq_m) + sq_n
                        t1 = t1pool.tile([128, NTILE], F32)
                        nc.vector.scalar_tensor_tensor(
                            t1[:], ps[:], sqm[:, m:m + 1],
                            sqb[:, q * QCOLS + n * NTILE:
                                q * QCOLS + (n + 1) * NTILE],
                            op0=ADD, op1=ADD,
                        )
                        # t2 = max(t1, 0)
                        t2 = t2pool.tile([128, NTILE], F32)
                        nc.vector.tensor_scalar_max(t2[:], t1[:], 0.0)
                        # orow tile = sqrt(t2)
                        nc.scalar.sqrt(orow[:, ns], t2[:])
                    nc.sync.dma_start(
                        out_d[m * 128:(m + 1) * 128,
                              q * QCOLS:(q + 1) * QCOLS],
                        orow[:],
                    )
    _split_excess_waits(nc, limit=1)
    return nc


_NC_CACHE = {}


def kernel(mapping: np.ndarray) -> np.ndarray:
    mapping = np.ascontiguousarray(mapping, dtype=np.float32)
    assert mapping.shape == (N, D)
    at = np.ascontiguousarray(mapping.T)                       # [D, N]
    sq = (
        np.einsum(
            "nd,nd->n", mapping.astype(np.float64), mapping.astype(np.float64)
        ).astype(np.float32)
    )
    sqr = sq.reshape(1, N)

    if "nc" not in _NC_CACHE:
        _NC_CACHE["nc"] = _build()
    nc = _NC_CACHE["nc"]

    in_maps = []
    for c in range(NCORES):
        lhs_c = np.ascontiguousarray(-2.0 * at[:, c * ROWS:(c + 1) * ROWS])
        sqm_c = np.ascontiguousarray(
            sq[c * ROWS:(c + 1) * ROWS].reshape(MT, 128).T
        )  # [128, MT]: [p, m] = sq[c*ROWS + m*128 + p]
        in_maps.append({"at": at, "lhs": lhs_c, "sqr": sqr, "sqm": sqm_c})

    res = run_bass_kernel_spmd(nc, in_maps, core_ids=list(range(NCORES)))
    return np.concatenate(
        [res.results[c]["out"] for c in range(NCORES)], axis=0
    )


# revision 4
# speedup vs baseline: 1.3220x; 1.0837x over previous
"""Pairwise Euclidean distance matrix on 8 Trainium2 NeuronCores.

Problem: mapping [8192, 512] f32 -> out[i,j] = ||mapping_i - mapping_j||_2,
shape [8192, 8192] f32.

Strategy (row/data parallel, per the sharding hint): core c computes output
rows [c*1024, (c+1)*1024). Since kernel() receives the full input on host,
each core is fed the full mapping directly (no on-device all-gather needed).

Math: d2 = sq_m + sq_n - 2*G with G = A_c @ A^T, then out = sqrt(max(d2, 0)).
The gram block is computed on TensorE in float32r (full-rate fp32 matmul,
~13-bit multiply mantissa - plenty here); the lhs operand is pre-scaled by -2
on host so PSUM accumulates -2G directly. DVE adds the two squared-norm terms
(per-partition scalar + broadcast row) and clamps at 0; ScalarE takes sqrt.

Layout: all DMA moves in >=8KB contiguous per-partition lines. A^T lives in
SBUF a quarter (2048 columns) at a time and doubles as the matmul rhs; output
is staged per (quarter, m-tile) in [128, 2048] row buffers.

Host-side prep (layout only): mapping^T contiguous (so the contraction dim
lands on SBUF partitions), the -2-scaled per-core lhs slice, and the row
norms sq laid out both as a [1, N] row and a per-core [128, MT] column table.
"""

import numpy as np
import bass_rust
import concourse.bass as bass
import concourse.mybir as mybir
from concourse.tile import TileContext
from concourse.bass_utils import run_bass_kernel_spmd

N = 8192          # points
D = 512           # dim
NCORES = 8
ROWS = N // NCORES        # 1024 output rows per core
MT = ROWS // 128          # 8 m-tiles (128 rows each)
NTILE = 512               # output columns per matmul (one PSUM bank)
KC = D // 128             # 4 contraction chunks of 128
QCOLS = 2048              # A^T columns resident per quarter
QT = N // QCOLS           # 4 quarters
QN = QCOLS // NTILE       # 4 n-tiles per quarter

F32 = mybir.dt.float32
F32R = mybir.dt.float32r
ADD = mybir.AluOpType.add


def _split_excess_waits(nc, limit=1):
    """The walrus build in this container rejects instructions carrying more
    than one sem-wait (e.g. fp32r Matmult S3_LW). Hoist excess waits onto
    same-engine NoOps inserted immediately before the instruction - waits
    execute in stream order on the engine's sequencer, so blocking semantics
    are identical."""
    for fn in nc.m.functions:
        for blk in fn.blocks:
            newlist = []
            changed = False
            for ins in blk.instructions:
                si = ins.sync_info
                if si is not None and si.on_wait and len(si.on_wait) > limit:
                    waits = list(si.on_wait)
                    excess, keep = waits[:-limit], waits[-limit:]
                    for i, w in enumerate(excess):
                        nop = bass_rust.InstNoOp(
                            name=f"{ins.name}-wsplit{i}", ins=[], outs=[]
                        )
                        nop.engine = ins.engine
                        nop.sync_info = mybir.SyncInfo(on_wait=[w], on_update=[])
                        newlist.append(nop)
                    si.on_wait = keep
                    ins.sync_info = si
                    changed = True
                newlist.append(ins)
            if changed:
                blk.instructions = newlist


def _build():
    nc = bass.Bass()
    at_d = nc.dram_tensor("at", [D, N], F32R, kind="ExternalInput")      # A^T
    lhs_d = nc.dram_tensor("lhs", [D, ROWS], F32R, kind="ExternalInput")  # -2*A_c^T
    sqr_d = nc.dram_tensor("sqr", [1, N], F32R, kind="ExternalInput")
    sqm_d = nc.dram_tensor("sqm", [128, MT], F32, kind="ExternalInput")
    ones_d = nc.dram_tensor("ones", [1, 128], F32R, kind="ExternalInput")
    out_d = nc.dram_tensor("out", [ROWS, N], F32, kind="ExternalOutput")

    with TileContext(nc) as tc:
        with (
            tc.tile_pool(name="const", bufs=1) as cpool,
            tc.tile_pool(name="atq", bufs=8) as apool,
            tc.tile_pool(name="sqbq", bufs=2) as bpool,
            tc.tile_pool(name="ps", bufs=4, space="PSUM") as pspool,
            tc.tile_pool(name="psb", bufs=4, space="PSUM") as psbpool,
            tc.tile_pool(name="t1", bufs=3) as t1pool,
            tc.tile_pool(name="t2", bufs=3) as t2pool,
            tc.tile_pool(name="orow", bufs=3) as opool,
        ):
            # Tiny constants first: sq row, per-m-tile sq column table, ones.
            sqr_sb = cpool.tile([1, N], F32R)
            nc.sync.dma_start(sqr_sb[:], sqr_d[:])
            sqm = cpool.tile([128, MT], F32)
            nc.sync.dma_start(sqm[:], sqm_d[:])
            ones = cpool.tile([1, 128], F32R)
            nc.sync.dma_start(ones[:], ones_d[:])

            # Resident -2*A_c^T chunks, one tile per 128-row contraction
            # chunk, interleaved with the first A^T quarter so the first
            # matmul group unblocks after ~1.5 MB of DMA.
            lhs = []
            for c in range(KC):
                lc = cpool.tile([128, ROWS], F32R, tag=f"lhs{c}")
                nc.sync.dma_start(lc[:], lhs_d[c * 128:(c + 1) * 128, :])
                lhs.append(lc)

            def load_quarter(q):
                atq = []
                for c in range(KC):
                    ac = apool.tile([128, QCOLS], F32R, tag="atq")
                    nc.sync.dma_start(
                        ac[:],
                        at_d[c * 128:(c + 1) * 128,
                             q * QCOLS:(q + 1) * QCOLS],
                    )
                    atq.append(ac)
                return atq

            atq_next = load_quarter(0)
            for q in range(QT):
                atq = atq_next
                # sq broadcast for this quarter, built on-chip:
                # ones[1,128].T @ sqr[1,512] -> PSUM -> SBUF (ScalarE copy).
                sqbq = bpool.tile([128, QCOLS], F32)
                for n in range(QN):
                    psb = psbpool.tile([128, NTILE], F32)
                    nc.tensor.matmul(
                        psb[:], ones[:],
                        sqr_sb[0:1, q * QCOLS + n * NTILE:
                               q * QCOLS + (n + 1) * NTILE],
                        start=True, stop=True,
                    )
                    nc.scalar.copy(sqbq[:, n * NTILE:(n + 1) * NTILE], psb[:])
                if q + 1 < QT:
                    atq_next = load_quarter(q + 1)
                for m in range(MT):
                    orow = opool.tile([128, QCOLS], F32)
                    for n in range(QN):
                        ns = slice(n * NTILE, (n + 1) * NTILE)
                        ps = pspool.tile([128, NTILE], F32)
                        for c in range(KC):
                            nc.tensor.matmul(
                                ps[:],
                                lhs[c][:, m * 128:(m + 1) * 128],
                                atq[c][:, ns],
                                start=(c == 0),
                                stop=(c == KC - 1),
                            )
                        # t1 = (-2G + sq_m) + sq_n
                        t1 = t1pool.tile([128, NTILE], F32)
                        nc.vector.scalar_tensor_tensor(
                            t1[:], ps[:], sqm[:, m:m + 1], sqbq[:, ns],
                            op0=ADD, op1=ADD,
                        )
                        # t2 = max(t1, 0)
                        t2 = t2pool.tile([128, NTILE], F32)
                        nc.vector.tensor_scalar_max(t2[:], t1[:], 0.0)
                        # orow tile = sqrt(t2)
                        nc.scalar.sqrt(orow[:, ns], t2[:])
                    nc.sync.dma_start(
                        out_d[m * 128:(m + 1) * 128,
                              q * QCOLS:(q + 1) * QCOLS],
                        orow[:],
                    )
    _split_excess_waits(nc, limit=1)
    return nc


_NC_CACHE = {}


def kernel(mapping: np.ndarray) -> np.ndarray:
    mapping = np.ascontiguousarray(mapping, dtype=np.float32)
    assert mapping.shape == (N, D)
    at = np.ascontiguousarray(mapping.T)                       # [D, N]
    sq = (
        np.einsum(
            "nd,nd->n", mapping.astype(np.float64), mapping.astype(np.float64)
        ).astype(np.float32)
    )
    sqr = sq.reshape(1, N)

    if "nc" not in _NC_CACHE:
        _NC_CACHE["nc"] = _build()
    nc = _NC_CACHE["nc"]

    in_maps = []
    for c in range(NCORES):
        lhs_c = np.ascontiguousarray(-2.0 * at[:, c * ROWS:(c + 1) * ROWS])
        sqm_c = np.ascontiguousarray(
            sq[c * ROWS:(c + 1) * ROWS].reshape(MT, 128).T
        )  # [128, MT]: [p, m] = sq[c*ROWS + m*128 + p]
        in_maps.append({"at": at, "lhs": lhs_c, "sqr": sqr, "sqm": sqm_c,
                        "ones": np.ones((1, 128), np.float32)})

    res = run_bass_kernel_spmd(nc, in_maps, core_ids=list(range(NCORES)))
    return np.concatenate(
        [res.results[c]["out"] for c in range(NCORES)], axis=0
    )
